# revision 15
# baseline (speedup 1.0000x reference)
"""Trainium2 Bass kernel for CanonicalAlignmentLoss.

Strategy ("subject-grouped sharding"):
  - Host groups the N=524288 rows by subject id (16 subjects) and deals each
    subject's rows round-robin-contiguously across the 8 cores, padding each
    (core, subject) segment with zero rows up to whole 128-row tiles so every
    SBUF tile is single-subject. A constant 1.0 column is appended so one
    matmul per tile yields both the gram block X^T X and the row-sum X^T 1.
  - Device (per core): stream the shard through SBUF in large contiguous
    chunks, and for each 128-row tile run one bf16 matmul accumulating into
    that subject's PSUM block.  bf16 inputs + fp32 PSUM accumulation give a
    loss rel-err of ~1e-5 (verified against the fp32 reference).
  - Host: sum the 8 per-core [64, 16*65] partials, form covariances, and do
    the tiny [16,16] pairwise-Frobenius stage.
"""

import numpy as np
import ml_dtypes

import concourse.bass as bass
import concourse.tile as tile
from concourse import bacc, mybir
from concourse.bass_utils import run_bass_kernel_spmd

NCORES = 8
S = 16
D = 64
LAM = 1e-4
ROWW = 66  # 64 data cols + 1 ones col + 1 pad col (keeps 4B-aligned tiles)
CTILE = 26  # 128-row tiles per DMA chunk (~440 KB chunks; minimal pad tiles)


def _build_nc(tiles_per_subject, nchunks, reps=1, bufs=8, altring=False):
    """Build the SPMD Bass program (identical on all cores).

    reps>1 repeats the whole compute schedule (each rep resets PSUM via
    start=True), used only for steady-state timing measurements.
    """
    nc = bacc.Bacc("TRN2", target_bir_lowering=False, debug=False)
    T = nchunks * CTILE

    x = nc.declare_dram_parameter(
        "x", [nchunks, 128, CTILE * ROWW], mybir.dt.bfloat16, isOutput=False
    )
    out = nc.declare_dram_parameter(
        "out", [64, S, 65], mybir.dt.float32, isOutput=True
    )

    # static schedule: subject for each 128-row tile + first/last flags
    sched = []
    for s, ts in enumerate(tiles_per_subject):
        for i in range(ts):
            sched.append((s, i == 0, i == ts - 1))
    assert len(sched) == T, (len(sched), T)

    with tile.TileContext(nc) as tc:
        with (
            tc.tile_pool(name="xin", bufs=bufs) as xpool,
            tc.tile_pool(name="ps", bufs=1, space=bass.MemorySpace.PSUM) as pspool,
            tc.tile_pool(name="osb", bufs=1) as opool,
        ):
            acc = pspool.tile([64, S, 128], mybir.dt.float32)
            osb = opool.tile([64, S, 65], mybir.dt.float32)
            for _rep in range(reps):
                for ch in range(nchunks):
                    xt = xpool.tile([128, CTILE * ROWW], mybir.dt.bfloat16)
                    eng = nc.scalar if (altring and ch % 2) else nc.sync
                    eng.dma_start(xt[:], x[ch])
                    for c in range(CTILE):
                        s, first, last = sched[ch * CTILE + c]
                        nc.tensor.matmul(
                            acc[0:64, s, 0:65],
                            xt[:, c * ROWW : c * ROWW + 64],
                            xt[:, c * ROWW : c * ROWW + 65],
                            start=first,
                            stop=last,
                        )
                        # drain each PSUM bank group (4 subject blocks) to
                        # SBUF as soon as its last accumulation lands, so
                        # only the final group's copy sits on the tail
                        if last and s % 4 == 3 and _rep == reps - 1:
                            g = s - 3
                            nc.vector.tensor_copy(
                                osb[:, g : g + 4, :], acc[:, g : g + 4, 0:65]
                            )
            nc.sync.dma_start(out[:], osb[:])
    nc.compile()
    return nc


def _prepare_shards(emb, sid):
    """Group rows by subject, shard across cores, pad to tiles, bf16-cast."""
    N = emb.shape[0]
    sid = np.asarray(sid).astype(np.int64).ravel()
    counts = np.bincount(sid, minlength=S).astype(np.int64)
    order = np.argsort(sid, kind="stable")
    starts = np.concatenate([[0], np.cumsum(counts)])

    # per-(core, subject) row counts: split n_s into 8 near-equal parts
    part = np.zeros((NCORES, S), np.int64)
    for s in range(S):
        q, r = divmod(int(counts[s]), NCORES)
        part[:, s] = q
        part[:r, s] += 1
    # tiles per subject: identical across cores (pad shorter parts with zeros)
    tiles_per_subject = [max(1, int(-(-int(part[:, s].max()) // 128))) for s in range(S)]
    T = sum(tiles_per_subject)
    nchunks = -(-T // CTILE)
    # pad the total tile count to a chunk multiple: extra all-zero tiles are
    # appended to subject 15's accumulation group (they contribute zero)
    tiles_per_subject[S - 1] += nchunks * CTILE - T
    T = nchunks * CTILE

    emb_bf = np.asarray(emb, dtype=np.float32).astype(ml_dtypes.bfloat16)

    tile_base = np.concatenate([[0], np.cumsum(tiles_per_subject)])
    shards = []
    for k in range(NCORES):
        arr = np.zeros((T * 128, ROWW), dtype=ml_dtypes.bfloat16)
        for s in range(S):
            off = int(starts[s] + part[:k, s].sum())
            n_ks = int(part[k, s])
            rows = order[off : off + n_ks]
            pos = int(tile_base[s]) * 128
            arr[pos : pos + n_ks, 0:D] = emb_bf[rows]
            arr[pos : pos + n_ks, D] = ml_dtypes.bfloat16(1.0)
        # chunk-partition-major layout: [nchunks, 128, CTILE*ROWW] where
        # dram[ch, p, c*ROWW+e] = row (ch*CTILE + c)*128 + p
        arr = np.ascontiguousarray(
            arr.reshape(nchunks, CTILE, 128, ROWW).transpose(0, 2, 1, 3)
        ).reshape(nchunks, 128, CTILE * ROWW)
        shards.append(arr)
    return shards, counts, tiles_per_subject, nchunks


def _finalize(partials, counts):
    """Reduce per-core stats and run the tiny [S,S] pairwise stage."""
    tot = np.zeros((64, S, 65), np.float64)
    for p in partials:
        tot += np.asarray(p, np.float64).reshape(64, S, 65)
    G = tot[:, :, 0:64].transpose(1, 0, 2)  # [S, 64, 64]
    sums = tot[:, :, 64].T  # [S, 64]
    n = counts.astype(np.float64)

    means = sums / np.maximum(n, 1.0)[:, None]
    denom = np.maximum(n - 1.0, 1.0)[:, None, None]
    cov = (G - n[:, None, None] * means[:, :, None] * means[:, None, :]) / denom
    # (+ LAM * I cancels in the pairwise differences, as in the reference)
    iu, ju = np.triu_indices(S, k=1)
    diff = cov[iu] - cov[ju]
    fro2 = np.sum(diff * diff, axis=(1, 2))
    valid = n >= 2.0
    pv = valid[iu] & valid[ju]
    vals = np.sqrt(np.where(pv, fro2, 1.0))
    total = np.sum(np.where(pv, vals, 0.0))
    cnt = int(pv.sum())
    loss = total / max(cnt, 1) if cnt > 0 else 0.0
    return np.float32(loss)


def kernel(embeddings, subject_ids):
    emb = np.asarray(embeddings)
    shards, counts, tiles_per_subject, nchunks = _prepare_shards(emb, subject_ids)
    nc = _build_nc(tiles_per_subject, nchunks)
    in_maps = [{"x": shards[k]} for k in range(NCORES)]
    res = run_bass_kernel_spmd(nc, in_maps, list(range(NCORES)))
    partials = [res.results[k]["out"] for k in range(NCORES)]
    return _finalize(partials, counts)
